# revision 46
# baseline (speedup 1.0000x reference)
"""BigBird encoder block on 8 Trainium2 NeuronCores — v2.

Sharding: pure data-parallel over batch (B=8 -> 1 batch element per core).

v2 changes vs baseline:
  * Attention matmuls run in fp8 with DoubleRow perf mode (0.5 cyc/col):
    - scores: per-(he, key-block) zero-padded DR (stationary [64,2,64] with
      a shared zero slot; moving q duplicated via a stride-0 axis).
    - AV: key-block PAIRS contracted in one DR matmul (stationary
      vkm[{j1,j2}] via step-sliced APs, moving att[{pos1,pos2}] via a
      constructed strided AP); singletons pair with a zero slot. The ones
      column (32.0) in every real vkm slot accumulates the softmax
      denominator for both paired blocks.
  * exp results live in a flat per-hp fp8 buffer (attflat) packed in
    emission order, so exp runs as few large [128,<=1024] ops.
  * rstd = exp(-0.5*ln(var+eps)) on the Act engine: Ln+Exp live in ONE
    activation table together with Copy/Square, so the whole kernel needs
    only 2 table loads (initial + gelu switch before the MLP).
  * LN stats via DVE tensor_reduce / tensor_tensor_reduce (off Act).
  * PSUM->SBUF copies batched to [128,512]; elementwise SBUF work
    (LN2 normalize, squares) moved to the idle Pool/GpSimd engine.
"""

import numpy as np
from contextlib import ExitStack

B, L, D = 8, 1024, 512
H, HD = 8, 64
BLK = 64
M = L // BLK   # 16
MLP = 1024
NCORES = 8
P = 128

_kernel_cache = {}


def _attended_sets(rand_attn):
    S = []
    for i in range(M):
        if i == 0 or i == M - 1:
            S.append(list(range(M)))
            continue
        s = {0, M - 1, (i - 1) % M, i, (i + 1) % M}
        for r in rand_attn[i]:
            s.add(int(r))
        S.append(sorted(s))
    return S


def _value_runs(vals):
    runs = []
    for x in vals:
        if runs and x == runs[-1][0] + runs[-1][1]:
            runs[-1][1] += 1
        else:
            runs.append([x, 1])
    return [(a, n) for a, n in runs]


def _plan_attention(rand_attn):
    """Host-side plan shared by every head pair.

    pieces:  [(j, chunk(list of query blocks), attcol)] in emission order.
    batches: groups of piece indices whose total width fits one
             [128, 1024] pssc tile.
    rects:   AV cover [(j1, j2_or_None, i0, nblk)] — middle blocks only;
             j=0 (start) and j=M-1 (stop) are emitted as fixed full-width
             singletons by the kernel.
    attpos:  (j, i) -> column of that (key block, query block) in attflat.
    """
    S = _attended_sets(rand_attn)
    Q = [[i for i in range(M) if j in S[i]] for j in range(M)]

    pieces = []
    attpos = {}
    chunkid = {}
    pos = 0
    for j in range(1, M - 1):
        qj = Q[j]
        for s0 in range(0, len(qj), 8):
            ch = qj[s0:s0 + 8]
            pieces.append((j, ch, pos))
            for r, i in enumerate(ch):
                attpos[(j, i)] = pos + 64 * r
                chunkid[(j, i)] = len(pieces) - 1
            pos += 64 * len(ch)
    ncols = pos

    batches = []
    cur, curlen = [], 0
    for pi, (j, ch, p0) in enumerate(pieces):
        w = 64 * len(ch)
        if curlen + w > 1024:
            batches.append(cur)
            cur, curlen = [], 0
        cur.append(pi)
        curlen += w
    if cur:
        batches.append(cur)

    def emit_rects(j1, j2, iset):
        out = []
        for (a, n) in _value_runs(sorted(iset)):
            cut = set()
            if a < 8 < a + n:
                cut.add(8)                       # acc PSUM bank boundary
            for i in range(a, a + n - 1):
                if chunkid[(j1, i)] != chunkid[(j1, i + 1)]:
                    cut.add(i + 1)
                if j2 is not None and chunkid[(j2, i)] != chunkid[(j2, i + 1)]:
                    cut.add(i + 1)
            pts = sorted(cut) + [a + n]
            prev = a
            for c in pts:
                if c > prev:
                    out.append((j1, j2, prev, c - prev))
                    prev = c
        return out

    rects = []
    U = {j: set(Q[j]) for j in range(1, M - 1)}
    while True:
        best, bl = None, 0
        for j1 in range(1, M - 1):
            for j2 in range(j1 + 1, M - 1):
                Lx = len(U[j1] & U[j2])
                if Lx > bl:
                    bl, best = Lx, (j1, j2)
        if best is None or bl == 0:
            break
        j1, j2 = best
        inter = U[j1] & U[j2]
        rects += emit_rects(j1, j2, inter)
        U[j1] -= inter
        U[j2] -= inter
    for j in range(1, M - 1):
        if U[j]:
            rects += emit_rects(j, None, U[j])

    return S, Q, pieces, batches, attpos, rects, ncols


def _np_bf16(x):
    import ml_dtypes
    return np.asarray(x, np.float32).astype(ml_dtypes.bfloat16)


def _np_fp8(x):
    import ml_dtypes
    return np.asarray(x, np.float32).astype(ml_dtypes.float8_e4m3)


def _build_bass(plan, bias1, bias2, pad_ones, b1zero=True):
    import bass_rust
    import concourse.bacc as bacc
    import concourse.tile as tile
    from concourse import mybir
    from concourse.masks import make_identity

    S, Q, pieces, batches, attpos, rects, NC_ATT = plan

    f32 = mybir.dt.float32
    bf16 = mybir.dt.bfloat16
    fp8 = mybir.dt.float8e4
    AF = mybir.ActivationFunctionType
    DR = mybir.MatmulPerfMode.DoubleRow
    ALU = mybir.AluOpType
    AX = mybir.AxisListType

    nc = bacc.Bacc("TRN2", target_bir_lowering=False)

    x_in = nc.dram_tensor("x_in", [L, D], bf16, kind="ExternalInput")
    wq_d = nc.dram_tensor("wq", [D, H * HD], fp8, kind="ExternalInput")
    wk_d = nc.dram_tensor("wk", [D, H * HD], fp8, kind="ExternalInput")
    wv_d = nc.dram_tensor("wv", [D, H * HD], fp8, kind="ExternalInput")
    wo_d = nc.dram_tensor("wo", [H * HD, D], fp8, kind="ExternalInput")
    w1_d = nc.dram_tensor("w1", [D, MLP], fp8, kind="ExternalInput")
    w2_d = nc.dram_tensor("w2", [MLP, D], fp8, kind="ExternalInput")
    b1_d = nc.dram_tensor("b1", [MLP], f32, kind="ExternalInput")
    b2_d = nc.dram_tensor("b2", [D], f32, kind="ExternalInput")
    if bias1:
        ln1b_d = nc.dram_tensor("ln1b", [D], f32, kind="ExternalInput")
    if not pad_ones:
        pad_d = nc.dram_tensor("padm", [P, M], f32, kind="ExternalInput")
    out_d = nc.dram_tensor("out", [L, D], bf16, kind="ExternalOutput")

    NT = L // P          # 8 token tiles
    DC = D // P          # 4
    HP = H // 2          # 4 head pairs
    MC = MLP // P        # 8
    EPS = 1e-6

    with tile.TileContext(nc) as tc, ExitStack() as ctx:
        const = ctx.enter_context(tc.tile_pool(name="const", bufs=1))
        big = ctx.enter_context(tc.tile_pool(name="big", bufs=1))
        resid = ctx.enter_context(tc.tile_pool(name="resid", bufs=3))
        tok = ctx.enter_context(tc.tile_pool(name="tok", bufs=6))
        small = ctx.enter_context(tc.tile_pool(name="small", bufs=6))
        rows = ctx.enter_context(tc.tile_pool(name="rows", bufs=2))
        attp = ctx.enter_context(tc.tile_pool(name="attp", bufs=4))
        bcast = ctx.enter_context(tc.tile_pool(name="bcast", bufs=4))
        normt = ctx.enter_context(tc.tile_pool(name="normt", bufs=4))
        # PSUM: two pools of 2 x 2-bank slots (8 banks total). psB's ring
        # is shared between the attention acc pairs and misc tiles of the
        # other phases; during attention ONLY acc tiles may come from psB
        # (a misc tile between an acc pair and its norm would deadlock the
        # ring on a forward dependency).
        psA = ctx.enter_context(tc.tile_pool(name="psA", bufs=2, space="PSUM"))
        psB = ctx.enter_context(tc.tile_pool(name="psB", bufs=2, space="PSUM"))

        _psn = [0]

        def psum(shape, dt, name=None, pool=0):
            pl = (psA, psB)[pool]
            if name is None:
                _psn[0] += 1
                name = f"ps{_psn[0]}"
            return pl.tile(shape, dt, tag="u" if pl is psA else "u2",
                           name=name)

        xts = []
        for t in range(NT):
            xt = tok.tile([P, D], bf16, tag="xt", bufs=NT, name=f"xt{t}")
            nc.sync.dma_start(xt[:], x_in[t * P:(t + 1) * P, :])
            xts.append(xt)

        id_bf = const.tile([P, P], bf16)
        make_identity(nc, id_bf[:])

        def load_w(dram, kdim, ndim):
            t = const.tile([P, kdim // P, ndim], fp8, tag=dram.name)
            nc.sync.dma_start(t[:], dram.rearrange("(c p) n -> p c n", p=P))
            return t

        wq = load_w(wq_d, D, H * HD)
        wk = load_w(wk_d, D, H * HD)
        wv = load_w(wv_d, D, H * HD)
        wo = load_w(wo_d, H * HD, D)
        w1 = load_w(w1_d, D, MLP)
        w2 = load_w(w2_d, MLP, D)

        b1c = const.tile([P, MC], f32)
        nc.sync.dma_start(b1c[:], b1_d.rearrange("(c p) -> p c", p=P))
        b2c = const.tile([P, DC], f32)
        nc.sync.dma_start(b2c[:], b2_d.rearrange("(c p) -> p c", p=P))
        if not pad_ones:
            padm = const.tile([P, M], f32)
            nc.sync.dma_start(padm[:], pad_d[:])
        if bias1:
            ln1bB = const.tile([P, D], f32)
            nc.sync.dma_start(ln1bB[:], ln1b_d[None, :].to_broadcast((P, D)))
        eps_col = const.tile([P, 1], f32)
        nc.vector.memset(eps_col[:], EPS)
        selb = const.tile([65, P], bf16)
        nc.vector.memset(selb[64:65, :], 1.0)
        onesb = const.tile([1, P], bf16)
        nc.vector.memset(onesb[:], 1.0)
        ones_colb = const.tile([P, 1], bf16)
        nc.vector.memset(ones_colb[:], 1.0)

        # big persistent activations
        xT = big.tile([P, DC, L], fp8, tag="xT")
        inT = resid.tile([P, DC, L], bf16, tag="res")
        qT = big.tile([P, HP, L], fp8, tag="qT")
        # kT2: [128, HP*M, 64] fp8; slot q = hp*M + j.
        kT2 = big.tile([P, HP * M, BLK], fp8, tag="kT2")
        # contiguous [K(0) | K(M-1)] per hp for the pair score stationary
        kT2p = big.tile([P, HP, 2 * BLK], fp8, tag="kT2p")
        vT = big.tile([P, HP, L], bf16, tag="vT")
        # vkm: per hp [128, M+2, 65] fp8; col 64 = ones (denominator).
        # slots M / M+1 = he0 / he1 stacked [V(0); V(M-1)] pair stationaries.
        vkm = big.tile([P, HP, M + 2, 65], fp8, tag="vkm")
        y1T = big.tile([P, MC, L], fp8, tag="y1T")
        outT = big.tile([P, HP, L], fp8, tag="outT")

        # ones columns (cheap, early; gpsimd is idle here)
        for hp in range(HP):
            nc.any.memset(vkm[:, hp, 0:M + 2, 64:65], 32.0)

        # ---- LN1 (token-major) + transposes ----
        def ln1_tile(t):
            xt = xts[t]
            pf = psum([P, D], bf16, name=f"tpb{t}")
            for c in range(DC):
                nc.tensor.transpose(pf[:, c * P:(c + 1) * P],
                                    xt[:, c * P:(c + 1) * P], id_bf[:])
            nc.any.tensor_copy(inT[:, :, t * P:(t + 1) * P],
                               pf[:].rearrange("p (c l) -> p c l", c=DC))
            st6 = small.tile([P, 6], f32, tag="st6")
            nc.vector.bn_stats(st6[:], xt[:])
            mv = small.tile([P, 2], f32, tag="mv")
            nc.vector.bn_aggr(mv[:], st6[:])
            rstd = small.tile([P, 1], f32, tag="rstd")
            nc.scalar.activation(rstd[:], mv[:, 1:2], AF.Sqrt,
                                 bias=eps_col[:])
            nc.vector.reciprocal(rstd[:], rstd[:])
            nmr = small.tile([P, 1], f32, tag="nmr")
            nc.vector.scalar_tensor_tensor(
                out=nmr[:], in0=mv[:, 0:1], scalar=-1.0, in1=rstd[:],
                op0=ALU.mult, op1=ALU.mult)
            xnb = tok.tile([P, D], bf16, tag="xnb")
            with nc.allow_low_precision(reason="qkv in fp8: rel tol 2e-2"):
                if bias1:
                    xn = tok.tile([P, D], f32, tag="xn")
                    nc.vector.tensor_scalar(
                        xn[:], xt[:], rstd[:], nmr[:],
                        op0=ALU.mult, op1=ALU.add)
                    nc.vector.tensor_add(xnb[:], xn[:], ln1bB[:])
                else:
                    nc.vector.tensor_scalar(
                        xnb[:], xt[:], rstd[:], nmr[:],
                        op0=ALU.mult, op1=ALU.add)
            pt = psum([P, D], bf16, name=f"tpa{t}", pool=1)
            for c in range(DC):
                nc.tensor.transpose(pt[:, c * P:(c + 1) * P],
                                    xnb[:, c * P:(c + 1) * P], id_bf[:])
            with nc.allow_low_precision(reason="qkv in fp8"):
                nc.any.tensor_copy(xT[:, :, t * P:(t + 1) * P],
                                   pt[:].rearrange("p (c l) -> p c l", c=DC))

        for t in range(NT):
            ln1_tile(t)

        # ---- QKV (fp8 DoubleRow over D=512 as 2 groups of 128x2) ----
        def qkv(w_sb, kind):
            for hp in range(HP):
                for nh in range(2):
                    ps = psum([P, 512], f32, pool=(hp + nh) % 2)
                    for c2 in range(DC // 2):
                        nc.tensor.matmul(
                            ps[:],
                            w_sb[:, 2 * c2:2 * c2 + 2, hp * P:(hp + 1) * P],
                            xT[:, 2 * c2:2 * c2 + 2, nh * 512:(nh + 1) * 512],
                            start=(c2 == 0), stop=(c2 == DC // 2 - 1),
                            perf_mode=DR)
                    sl = slice(nh * 512, (nh + 1) * 512)
                    with nc.allow_low_precision(reason="attn fp8"):
                        if kind == "v":
                            nc.any.tensor_copy(vT[:, hp, sl], ps[:])
                        elif kind == "k":
                            dst = kT2[:, hp * M + nh * 8: hp * M + nh * 8 + 8, :]
                            dst = dst.rearrange("p j c -> p (j c)")
                            nc.any.tensor_copy(dst, ps[:])
                            if nh == 0:
                                nc.any.tensor_copy(
                                    kT2p[:, hp, 0:BLK], ps[:, 0:BLK])
                            else:
                                nc.any.tensor_copy(
                                    kT2p[:, hp, BLK:2 * BLK],
                                    ps[:, 512 - BLK:512])
                        else:
                            nc.any.tensor_copy(qT[:, hp, sl], ps[:])

        qkv(wk, "k")
        qkv(wq, "q")
        qkv(wv, "v")

        # ---- vkm prebuild (V transposed to keytok-on-partitions) ----
        for hp in range(HP):
            pt = psum([P, M + 2, 64], bf16, name=f"tpv{hp}", pool=hp % 2)
            for j in range(M):
                nc.tensor.transpose(pt[0:64, j, :],
                                    vT[0:64, hp, j * 64:(j + 1) * 64],
                                    id_bf[0:64, 0:64])
                nc.tensor.transpose(pt[64:128, j, :],
                                    vT[64:128, hp, j * 64:(j + 1) * 64],
                                    id_bf[64:128, 64:128])
            # pair stacks: [V(0) top; V(M-1) bottom] per he (out base free)
            nc.tensor.transpose(pt[0:64, M, :], vT[0:64, hp, 0:64],
                                id_bf[0:64, 0:64])
            nc.tensor.transpose(pt[64:128, M, :],
                                vT[0:64, hp, (M - 1) * 64:M * 64],
                                id_bf[0:64, 0:64])
            nc.tensor.transpose(pt[0:64, M + 1, :], vT[64:128, hp, 0:64],
                                id_bf[64:128, 64:128])
            nc.tensor.transpose(pt[64:128, M + 1, :],
                                vT[64:128, hp, (M - 1) * 64:M * 64],
                                id_bf[64:128, 64:128])
            with nc.allow_low_precision(reason="v fp8"):
                nc.any.tensor_copy(vkm[:, hp, 0:M + 2, 0:64], pt[:])
            if not pad_ones:
                for j in range(M):
                    nc.gpsimd.tensor_scalar_mul(vkm[:, hp, j, :],
                                                vkm[:, hp, j, :],
                                                padm[:, j:j + 1])

        # ---- attention ----
        def emit_scores(hp):
            """Pair (0, M-1) scores + middle score matmuls + exp."""
            attP = attPs[hp]
            for he in range(2):
                po = he * 64
                psscP = psum([P, 1024], f32, name=f"scP{hp}_{he}")
                st = kT2p[po:po + 64, hp, :]
                for half in range(2):
                    nc.tensor.matmul(
                        psscP[:, half * 512:(half + 1) * 512],
                        st, qT[po:po + 64, hp,
                               half * 512:(half + 1) * 512],
                        start=True, stop=True)
                with nc.allow_low_precision(reason="att fp8"):
                    nc.scalar.activation(attP[:, he, :], psscP[:], AF.Exp,
                                         scale=2.0 ** -10)
            att = attflat[hp]
            for bi, batch in enumerate(batches):
                b0 = pieces[batch[0]][2]            # attflat col of batch
                blen = sum(64 * len(pieces[pi][1]) for pi in batch)
                pssc = psum([P, 1024], f32, name=f"sc{hp}_{bi}")
                for pi in batch:
                    j, ch, p0 = pieces[pi]
                    rel = p0 - b0
                    # runs of consecutive query blocks, split at the pssc
                    # bank boundary (columns 512)
                    col = rel
                    for (i0, n) in _value_runs(ch):
                        c = col
                        left = n
                        a = i0
                        while left:
                            room = (512 - c % 512) // 64 if c % 512 else 8
                            take = min(left, room if room else 8)
                            kslot = hp * M + j
                            for he in range(2):
                                po = he * 64
                                nc.tensor.matmul(
                                    pssc[po:po + 64, c:c + take * 64],
                                    kT2[po:po + 64, kslot, :],
                                    qT[po:po + 64, hp,
                                       a * 64:(a + take) * 64],
                                    start=True, stop=True)
                            c += take * 64
                            a += take
                            left -= take
                        col += n * 64
                with nc.allow_low_precision(reason="att fp8"):
                    nc.scalar.activation(att[:, b0:b0 + blen],
                                         pssc[:, 0:blen], AF.Exp,
                                         scale=2.0 ** -10)

        def emit_av(hp, acc):
            """Pair (0, M-1) AV starts every column at full 128-depth;
            middle pieces accumulate; stop flag on the last piece
            (sim metadata only — no HW effect)."""
            attP = attPs[hp]
            att = attflat[hp]
            for he in range(2):
                for half in range(2):
                    nc.tensor.matmul(
                        acc[he][0:65, half * 512:(half + 1) * 512],
                        vkm[0:128, hp, M + he, :],
                        attP[0:128, he, half * 512:(half + 1) * 512],
                        start=True, stop=False)
            lastj = pieces[-1][0]
            for (j, ch, p0) in pieces:
                col = p0
                for (i0, n) in _value_runs(ch):
                    c = col
                    left = n
                    a = i0
                    while left:
                        take = min(left, 8 - a % 8 if a % 8 else 8)
                        for he in range(2):
                            po = he * 64
                            nc.tensor.matmul(
                                acc[he][0:65, a * 64:(a + take) * 64],
                                vkm[po:po + 64, hp, j, :],
                                att[po:po + 64, c:c + take * 64],
                                start=False, stop=(j == lastj))
                        c += take * 64
                        a += take
                        left -= take
                    col += n * 64

        def make_norm(hp, acc):
            tail = (hp == HP - 1)
            def norm():
                recm = [rows.tile([65, L], bf16, tag=f"rec{he}",
                                  name=f"rec{hp}_{he}") for he in range(2)]
                # half-row recips: the first recB waits 596ns, not 2.4us
                with nc.allow_low_precision(reason="1/denom bf16"):
                    for nh0 in range(2):
                        sl0 = slice(nh0 * 512, (nh0 + 1) * 512)
                        nc.vector.reciprocal(recm[0][64:65, sl0],
                                             acc[0][64:65, sl0])
                        nc.vector.reciprocal(recm[1][64:65, sl0],
                                             acc[1][64:65, sl0])
                nt = normt.tile([64, L], fp8, tag="nt")
                for nh in range(2):
                    sl = slice(nh * 512, (nh + 1) * 512)
                    for he in range(2):
                        recB = psum([64, 512], f32,
                                    name=f"recB{hp}_{nh}_{he}", pool=0)
                        nc.tensor.matmul(recB[:], selb[64:65, 0:64],
                                         recm[he][64:65, sl],
                                         start=True, stop=True)
                        recS = bcast.tile([64, 512], f32, tag="recB")
                        nc.any.tensor_copy(recS[:], recB[:])
                        with nc.allow_low_precision(reason="attn out fp8"):
                            if he == 0:
                                nc.any.tensor_mul(outT[0:64, hp, sl],
                                                  acc[0][0:64, sl],
                                                  recS[:])
                            else:
                                nc.any.tensor_mul(nt[:, sl],
                                                  acc[1][0:64, sl],
                                                  recS[:])
                    nc.sync.dma_start(outT[64:128, hp, sl], nt[:, sl])
            return norm

        attflat = [attp.tile([P, NC_ATT], fp8, tag="attflat",
                             name=f"attflat{hp}") for hp in range(HP)]
        attPs = [attp.tile([P, 2, 1024], fp8, tag="attP",
                           name=f"attP{hp}") for hp in range(HP)]

        # emission: scores(0),scores(1),AV(0),scores(2),norm(0),AV(1),
        # scores(3),norm(1),AV(2),norm(2),AV(3),norm(3)
        accs = {}
        pending_norm = None
        for hp in range(HP):
            emit_scores(hp)
            if hp >= 1:
                prev = hp - 1
                accs[prev] = [psB.tile([65, L], f32, tag="u2",
                                       name=f"acc{prev}_{he}")
                              for he in range(2)]
                emit_av(prev, accs[prev])
                if pending_norm is not None:
                    pending_norm()
                pending_norm = make_norm(prev, accs[prev])
        accs[HP - 1] = [psB.tile([65, L], f32, tag="u2",
                                 name=f"acc{HP - 1}_{he}")
                        for he in range(2)]
        emit_av(HP - 1, accs[HP - 1])
        if pending_norm is not None:
            pending_norm()
        make_norm(HP - 1, accs[HP - 1])()

        # ---- Wo + residual + LN2 stats, interleaved across nh ----
        xrT = resid.tile([P, DC, L], bf16, tag="res")
        mu = rows.tile([1, L], f32, tag="mu")
        msq = rows.tile([1, L], f32, tag="msq")
        mub = rows.tile([1, L], bf16, tag="mub")
        msqb = rows.tile([1, L], bf16, tag="msqb")

        def emit_wo(nh):
            for dc in range(DC):
                sl = slice(nh * 512, (nh + 1) * 512)
                ps = psum([P, 512], f32, pool=dc % 2)
                for c2 in range(HP // 2):
                    nc.tensor.matmul(
                        ps[:],
                        wo[:, 2 * c2:2 * c2 + 2, dc * P:(dc + 1) * P],
                        outT[:, 2 * c2:2 * c2 + 2, sl],
                        start=(c2 == 0), stop=(c2 == HP // 2 - 1),
                        perf_mode=DR)
                with nc.allow_low_precision(reason="xr bf16: tol 2e-2"):
                    nc.vector.scalar_tensor_tensor(
                        out=xrT[:, dc, sl], in0=ps[:], scalar=2.0 ** -5,
                        in1=inT[:, dc, sl],
                        op0=ALU.mult, op1=ALU.add)

        def emit_ln2_stats(nh):
            ps_s = psum([1, 512], f32, name=f"st_s{nh}", pool=0)
            ps_q = psum([1, 512], f32, name=f"st_q{nh}", pool=1)
            for dc in range(DC):
                sl = slice(nh * 512, (nh + 1) * 512)
                sqc = tok.tile([P, 512], bf16, tag="sqc")
                with nc.allow_low_precision(reason="ln2 stats in bf16"):
                    nc.any.tensor_mul(sqc[:], xrT[:, dc, sl],
                                      xrT[:, dc, sl])
                nc.tensor.matmul(ps_s[:], ones_colb[:], xrT[:, dc, sl],
                                 start=(dc == 0), stop=(dc == DC - 1))
                nc.tensor.matmul(ps_q[:], ones_colb[:], sqc[:],
                                 start=(dc == 0), stop=(dc == DC - 1))
            sl = slice(nh * 512, (nh + 1) * 512)
            with nc.allow_low_precision(reason="ln2 mean row bf16"):
                nc.scalar.mul(mub[0:1, sl], ps_s[:], 1.0 / D)
            nc.scalar.mul(mu[0:1, sl], ps_s[:], 1.0 / D)
            mu2 = rows.tile([1, L], f32, tag="mu2")
            nc.any.tensor_mul(mu2[0:1, sl], mu[0:1, sl], mu[0:1, sl])
            # var = ps_q/D - mu^2 in one fused op
            nc.vector.scalar_tensor_tensor(
                out=msq[0:1, sl], in0=ps_q[:], scalar=1.0 / D,
                in1=mu2[0:1, sl], op0=ALU.mult, op1=ALU.subtract)
            nc.scalar.activation(msq[0:1, sl], msq[0:1, sl], AF.Sqrt,
                                 bias=eps_col[0:1, :])
            with nc.allow_low_precision(reason="ln2 stat rows bf16"):
                nc.vector.reciprocal(msqb[0:1, sl], msq[0:1, sl])

        emit_wo(0)
        emit_wo(1)
        emit_ln2_stats(0)
        emit_ln2_stats(1)
        ln2T = big.tile([P, DC, L], fp8, tag="ln2T")
        for nh in range(2):
            sl = slice(nh * 512, (nh + 1) * 512)
            muB = psum([P, 512], f32, name=f"muB{nh}", pool=0)
            rstdB = psum([P, 512], f32, name=f"rstdB{nh}", pool=1)
            nc.tensor.matmul(muB[:], onesb[0:1, :], mub[0:1, sl],
                             start=True, stop=True)
            nc.tensor.matmul(rstdB[:], onesb[0:1, :], msqb[0:1, sl],
                             start=True, stop=True)
            muS = bcast.tile([P, 512], bf16, tag="muS")
            rstdS = bcast.tile([P, 512], bf16, tag="rstdS")
            with nc.allow_low_precision(reason="ln2 bcast bf16"):
                nc.any.tensor_copy(muS[:], muB[:])
                nc.any.tensor_copy(rstdS[:], rstdB[:])
            for dc in range(DC):
                t1 = tok.tile([P, 512], bf16, tag="sqc")
                with nc.allow_low_precision(reason="mlp in fp8"):
                    nc.any.tensor_sub(t1[:], xrT[:, dc, sl], muS[:])
                    nc.any.tensor_mul(ln2T[:, dc, sl], t1[:], rstdS[:])

        # ---- MLP ----
        foutT = resid.tile([P, DC, L], bf16, tag="res")
        dmaq = [nc.sync, nc.sync, nc.sync]

        def emit_mlp1(nh, mc):
            ps = psum([P, 512], f32, pool=(mc + nh) % 2)
            for c2 in range(DC // 2):
                nc.tensor.matmul(
                    ps[:], w1[:, 2 * c2:2 * c2 + 2, mc * P:(mc + 1) * P],
                    ln2T[:, 2 * c2:2 * c2 + 2, nh * 512:(nh + 1) * 512],
                    start=(c2 == 0), stop=(c2 == DC // 2 - 1),
                    perf_mode=DR)
            with nc.allow_low_precision(reason="mlp hidden fp8"):
                nc.scalar.activation(y1T[:, mc, nh * 512:(nh + 1) * 512],
                                     ps[:], AF.Gelu_apprx_tanh,
                                     bias=b1c[:, mc:mc + 1],
                                     scale=2.0 ** -5)

        def emit_mlp1_pair(nh, mcp):
            # zero-bias specialization: two mc units share one [128,1024]
            # psum (one bank each) and ONE batched gelu
            ps = psum([P, 1024], f32, pool=(mcp // 2 + nh) % 2)
            for k in range(2):
                mc = mcp + k
                for c2 in range(DC // 2):
                    nc.tensor.matmul(
                        ps[:, k * 512:(k + 1) * 512],
                        w1[:, 2 * c2:2 * c2 + 2, mc * P:(mc + 1) * P],
                        ln2T[:, 2 * c2:2 * c2 + 2,
                             nh * 512:(nh + 1) * 512],
                        start=(c2 == 0), stop=(c2 == DC // 2 - 1),
                        perf_mode=DR)
            with nc.allow_low_precision(reason="mlp hidden fp8"):
                nc.scalar.activation(
                    y1T[:, mcp:mcp + 2, nh * 512:(nh + 1) * 512],
                    ps[:].rearrange("p (k l) -> p k l", k=2),
                    AF.Gelu_apprx_tanh, scale=2.0 ** -5)

        def emit_mlp2(nh, dc):
            ps = psum([P, 512], f32, pool=(dc + nh) % 2)
            for c2 in range(MC // 2):
                nc.tensor.matmul(
                    ps[:], w2[:, 2 * c2:2 * c2 + 2, dc * P:(dc + 1) * P],
                    y1T[:, 2 * c2:2 * c2 + 2, nh * 512:(nh + 1) * 512],
                    start=(c2 == 0), stop=(c2 == MC // 2 - 1),
                    perf_mode=DR)
            sl = slice(nh * 512, (nh + 1) * 512)
            with nc.allow_low_precision(reason="final out bf16"):
                if bias2:
                    t2 = tok.tile([P, 512], f32, tag="sqc",
                                  name=f"t2_{dc}_{nh}")
                    nc.vector.tensor_scalar(
                        t2[:], ps[:], 2.0 ** -5, b2c[:, dc:dc + 1],
                        op0=ALU.mult, op1=ALU.add)
                    nc.vector.tensor_add(foutT[:, dc, sl], t2[:],
                                         xrT[:, dc, sl])
                else:
                    nc.vector.scalar_tensor_tensor(
                        out=foutT[:, dc, sl], in0=ps[:],
                        scalar=2.0 ** -5, in1=xrT[:, dc, sl],
                        op0=ALU.mult, op1=ALU.add)

        for nh in range(2):
            if b1zero:
                for mcp in range(0, MC, 2):
                    emit_mlp1_pair(nh, mcp)
            else:
                for mc in range(MC):
                    emit_mlp1(nh, mc)
            for dc in range(DC):
                emit_mlp2(nh, dc)
            pfs = [psum([P, D], bf16, name=f"tpo{t}", pool=t % 2)
                   for t in range(nh * NT // 2, (nh + 1) * NT // 2)]
            for dc in range(DC):
                for ti, t in enumerate(range(nh * NT // 2,
                                             (nh + 1) * NT // 2)):
                    nc.tensor.transpose(pfs[ti][:, dc * P:(dc + 1) * P],
                                        foutT[:, dc, t * P:(t + 1) * P],
                                        id_bf[:])
            for ti, t in enumerate(range(nh * NT // 2, (nh + 1) * NT // 2)):
                ost = normt.tile([P, D], bf16, tag="ost")
                nc.any.tensor_copy(ost[:], pfs[ti][:])
                dmaq[t % 3].dma_start(out_d[t * P:(t + 1) * P, :], ost[:])

    nc.compile()
    return nc


def kernel(**inputs):
    inputs = {k: np.asarray(v) for k, v in inputs.items()}
    rand_attn = inputs["rand_attn"].astype(np.int32)
    ln1s = inputs["ln1_scale"].astype(np.float32)
    ln1b = inputs["ln1_bias"].astype(np.float32)
    ln2s = inputs["ln2_scale"].astype(np.float32)
    ln2b = inputs["ln2_bias"].astype(np.float32)
    pm = np.asarray(inputs["padding_mask"]).astype(np.float32)
    bias1 = bool(np.any(ln1b != 0.0))
    bias2 = bool(np.any(np.asarray(inputs["b2"]) != 0.0))
    pad_ones = bool(np.all(pm == 1.0))
    w1f_pre = inputs["W1"].astype(np.float32)
    b1f_pre = inputs["b1"].astype(np.float32) + ln2b @ w1f_pre
    b1zero = not bool(np.any(b1f_pre != 0.0))
    key = (rand_attn.tobytes(), bias1, bias2, pad_ones, b1zero)
    if key not in _kernel_cache:
        plan = _plan_attention(rand_attn)
        _kernel_cache[key] = _build_bass(plan, bias1, bias2, pad_ones,
                                         b1zero)
    nc = _kernel_cache[key]

    x = inputs["inputs"].astype(np.float32)
    s1 = np.where(ln1s == 0.0, np.float32(1e-30), ln1s)[:, None]
    wq = _np_fp8(32.0 * s1 * inputs["Wq"].reshape(D, H * HD) / np.sqrt(HD))
    wk = _np_fp8(32.0 * s1 * inputs["Wk"].reshape(D, H * HD))
    wv = _np_fp8(32.0 * s1 * inputs["Wv"].reshape(D, H * HD))
    wo = _np_fp8(32.0 * inputs["Wo"].reshape(H * HD, D))
    w1f = inputs["W1"].astype(np.float32)
    w1 = _np_fp8(32.0 * ln2s[:, None] * w1f)
    w2 = _np_fp8(32.0 * inputs["W2"])
    b1f = inputs["b1"].astype(np.float32) + ln2b @ w1f
    common = dict(
        wq=wq, wk=wk, wv=wv, wo=wo, w1=w1, w2=w2,
        b1=b1f.astype(np.float32),
        b2=inputs["b2"].astype(np.float32),
    )
    if bias1:
        common["ln1b"] = (ln1b / s1[:, 0]).astype(np.float32)
    in_maps = []
    for c in range(NCORES):
        m = dict(common, x_in=_np_bf16(x[c]))
        if not pad_ones:
            pj = pm[c, :, 0].reshape(M, BLK).T
            m["padm"] = np.concatenate([pj, pj], axis=0).astype(np.float32)
        in_maps.append(m)

    from concourse.bass_utils import run_bass_kernel_spmd
    res = run_bass_kernel_spmd(nc, in_maps, core_ids=list(range(NCORES)))
    return np.stack([np.asarray(res.results[c]["out"], np.float32)
                     for c in range(NCORES)], axis=0)
